# revision 7
# baseline (speedup 1.0000x reference)
"""Trainium2 8-core kernel for nn_AdaptiveMask (segment_reduce).

reference semantics:
    h = l2norm(head_embeds); t = l2norm(tail_embeds)
    edge_alpha = (sum(h*t, -1) + 1) * 0.5                       # [E]
    row_sum    = segment_sum(edge_alpha, head_list, N)          # [N]
    d_inv      = where(row_sum != 0, 1/row_sum, 0)
    G_indices  = stack([head_list, tail_list])                  # [2, E] i32
    G_values   = d_inv[head_list] * edge_alpha                  # [E] f32

Strategy (sharding = sort-pack permutation, inverted on output):
  * Host sorts edges by head (stable), packs whole segments (runs of equal
    head) into 8*128 partition rows of fixed capacity K, padding row tails
    with phantom edges.  No segment crosses a row, so every core/partition
    reduces its segments independently -- no collective needed.
  * Device (per core, SPMD identical graph):
      - DMA-cast embeds f32->bf16, chunked [128, C*64]
      - DVE: m_ht = h*t (bf16 2x); ACT: m_hh = h^2, m_tt = t^2
      - fold-pairs (bf16 2x adds) + tensor_reduce -> per-edge dots [128, C]
      - alpha = 0.5 + 0.5 * dht * rsqrt(dhh*dtt)   (ACT Sqrt + DVE recip)
      - segment masks from not_equal(head[e], head[e-1])
      - segmented prefix-sum scan + reversed hold scan (tensor_tensor_scan)
        give V[e] = row_sum[head_e] per edge, entirely in-SBUF
      - G[e] = alpha[e] * reciprocal(V[e])
  * Host scatters device output back to original edge order.
"""

import sys

for _p in ("/opt/trn_rl_repo",):
    if _p not in sys.path:
        sys.path.insert(0, _p)

import numpy as np

# ---------------------------------------------------------------- constants
E = 3_200_000
D = 64
N_NODES = 100_000
NCORES = 8
P = 128                 # SBUF partitions
K = 3360                # edge slots per partition row  (35 chunks of 96)
C = 96                  # edges per partition per chunk
ROWS = NCORES * P       # 1024 packing rows
EPC = P * K             # padded edges per core


def build_graph(p=P, k=K, c=C, d=D, repeat=1):
    """Build the single-core Bass graph (SPMD: all cores run this).

    repeat>1 wraps the whole body in a For_i loop (for HW timing only).
    """
    import concourse.bacc as bacc
    import concourse.mybir as mybir
    import concourse.tile as tile
    from contextlib import nullcontext

    f32 = mybir.dt.float32
    bf16 = mybir.dt.bfloat16
    i32 = mybir.dt.int32
    Alu = mybir.AluOpType
    Act = mybir.ActivationFunctionType

    nc = bacc.Bacc(None, target_bir_lowering=False)

    h_in = nc.dram_tensor("h", [p * k, d], f32, kind="ExternalInput")
    t_in = nc.dram_tensor("t", [p * k, d], f32, kind="ExternalInput")
    head_in = nc.dram_tensor("head", [p * k], i32, kind="ExternalInput")
    g_out = nc.dram_tensor("g", [p * k], f32, kind="ExternalOutput")

    hv = h_in[:].rearrange("(p k) d -> p (k d)", p=p)      # [P, K*D]
    tv = t_in[:].rearrange("(p k) d -> p (k d)", p=p)
    headv = head_in[:].rearrange("(p k) -> p k", p=p)      # [P, K]
    gv = g_out[:].rearrange("(p k) -> p k", p=p)

    nchunks = k // c

    with tile.TileContext(nc) as tc:
        with (
            tc.tile_pool(name="persist", bufs=1) as pers,
            tc.tile_pool(name="io", bufs=2) as io,
            tc.tile_pool(name="mbuf", bufs=1) as mb,
            tc.tile_pool(name="small", bufs=2) as sm,
            tc.tile_pool(name="folds", bufs=1) as fp,
            tc.For_i(0, repeat, 1) if repeat > 1 else nullcontext(),
        ):
            alpha = pers.tile([p, k], f32)
            notm0 = pers.tile([p, k + 1], f32)   # notm0[e]=1 iff head[e]==head[e-1]
            ps = pers.tile([p, k], f32, tag="pg")
            vrev = pers.tile([p, k], f32)
            headt = pers.tile([p, k], i32, tag="hs")

            nc.sync.dma_start(out=headt[:], in_=headv)

            # segment-start masks: notm0[:,0]=0 (row start), sentinel notm0[:,K]=0
            nc.vector.tensor_tensor(
                out=notm0[:, 1:k], in0=headt[:, 1:k], in1=headt[:, 0 : k - 1],
                op=Alu.is_equal,
            )
            nc.vector.memset(notm0[:, 0:1], 0.0)
            nc.vector.memset(notm0[:, k : k + 1], 0.0)

            for ci in range(nchunks):
                sl = slice(ci * c * d, (ci + 1) * c * d)
                hb = io.tile([p, c * d], bf16, tag="hb")
                tb = io.tile([p, c * d], bf16, tag="tb")
                nc.gpsimd.dma_start(out=hb[:], in_=hv[:, sl])   # f32->bf16 cast
                nc.gpsimd.dma_start(out=tb[:], in_=tv[:, sl])

                mht = mb.tile([p, c * d], bf16, tag="mht")
                mhh = mb.tile([p, c * d], bf16, tag="mhh")
                mtt = mb.tile([p, c * d], bf16, tag="mtt")
                nc.scalar.activation(mhh[:], hb[:], Act.Square)
                nc.scalar.activation(mtt[:], tb[:], Act.Square)
                nc.vector.tensor_mul(mht[:], hb[:], tb[:])

                reds = {}
                for nm, m in (("hh", mhh), ("tt", mtt), ("ht", mht)):
                    m3 = m[:].rearrange("p (c d) -> p c d", d=d)
                    f1 = fp.tile([p, c, d // 2], bf16, tag="f1")
                    nc.vector.tensor_add(
                        f1[:], m3[:, :, 0 : d // 2], m3[:, :, d // 2 : d]
                    )
                    f2 = fp.tile([p, c, d // 4], bf16, tag="f2")
                    nc.vector.tensor_add(
                        f2[:], f1[:, :, 0 : d // 4], f1[:, :, d // 4 : d // 2]
                    )
                    red = sm.tile([p, c], f32, tag=f"red{nm}")
                    nc.vector.tensor_reduce(
                        red[:], f2[:], axis=mybir.AxisListType.X, op=Alu.add
                    )
                    reds[nm] = red

                q = sm.tile([p, c], f32, tag="q")
                nc.vector.tensor_mul(q[:], reds["hh"][:], reds["tt"][:])
                s = sm.tile([p, c], f32, tag="s")
                nc.scalar.activation(s[:], q[:], Act.Sqrt)
                r = sm.tile([p, c], f32, tag="r")
                nc.vector.reciprocal(r[:], s[:])
                tmp = sm.tile([p, c], f32, tag="tmp")
                nc.vector.tensor_mul(tmp[:], reds["ht"][:], r[:])
                nc.vector.tensor_scalar(
                    out=alpha[:, ci * c : (ci + 1) * c], in0=tmp[:],
                    scalar1=0.5, scalar2=0.5, op0=Alu.mult, op1=Alu.add,
                )

            # ps[e] = notm0[e]*ps[e-1] + alpha[e]  (segmented prefix sum)
            nc.vector.tensor_tensor_scan(
                out=ps[:], data0=notm0[:, 0:k], data1=alpha[:],
                initial=0.0, op0=Alu.mult, op1=Alu.add,
            )
            # stamp[e] = ps[e] * m1[e],  m1[e] = 1 - notm0[e+1]  (segment ends)
            stamp = pers.tile([p, k], f32, tag="hs")
            nc.vector.tensor_mul(stamp[:], ps[:], notm0[:, 1 : k + 1])
            nc.vector.tensor_sub(stamp[:], ps[:], stamp[:])
            # reversed hold scan: V[e] = segment total broadcast over segment
            nc.vector.tensor_tensor_scan(
                out=vrev[:], data0=notm0[:, k:0:-1], data1=stamp[:, ::-1],
                initial=0.0, op0=Alu.mult, op1=Alu.add,
            )
            nc.vector.reciprocal(vrev[:], vrev[:])
            gtile = pers.tile([p, k], f32, tag="pg")
            nc.vector.tensor_mul(gtile[:], alpha[:], vrev[:, ::-1])
            nc.sync.dma_start(out=gv, in_=gtile[:])

    nc.finalize()
    return nc


# ---------------------------------------------------------------- host side

def _pack(head_list):
    """Sort edges by head; pack whole segments into ROWS rows of capacity K.

    Returns (order, slots) where order[i] = original edge id of i-th sorted
    edge and slots[i] = its slot in the padded [ROWS*K] device layout.
    """
    order = np.argsort(head_list, kind="stable")
    hs = head_list[order]
    # segment boundaries
    change = np.flatnonzero(hs[1:] != hs[:-1]) + 1
    seg_starts = np.concatenate([[0], change])
    seg_sizes = np.diff(np.concatenate([seg_starts, [len(hs)]]))
    cum = np.cumsum(seg_sizes)
    nseg = len(seg_sizes)
    assert seg_sizes.max() <= K, f"segment too large: {seg_sizes.max()}"

    counts = np.zeros(ROWS, dtype=np.int64)
    base = 0
    j0 = 0
    for rr in range(ROWS):
        j = int(np.searchsorted(cum, base + K, side="right"))
        if j > j0:
            counts[rr] = cum[j - 1] - base
            base = cum[j - 1]
            j0 = j
        if j0 >= nseg:
            break
    assert j0 >= nseg, f"packing overflow: {j0}/{nseg} segments placed"

    off = np.concatenate([[0], np.cumsum(counts)[:-1]])
    row_of = np.repeat(np.arange(ROWS), counts)
    pos_in_row = np.arange(len(hs)) - np.repeat(off, counts)
    slots = row_of * K + pos_in_row
    return order, slots


_CACHE = {}


def kernel(head_embeds, tail_embeds, head_list, tail_list):
    from concourse.bass_utils import run_bass_kernel_spmd

    head_embeds = np.asarray(head_embeds, dtype=np.float32)
    tail_embeds = np.asarray(tail_embeds, dtype=np.float32)
    head_list = np.asarray(head_list, dtype=np.int32)
    tail_list = np.asarray(tail_list, dtype=np.int32)

    order, slots = _pack(head_list)

    h_pad = np.ones((ROWS * K, D), dtype=np.float32)
    t_pad = np.ones((ROWS * K, D), dtype=np.float32)
    head_pad = np.full(ROWS * K, N_NODES, dtype=np.int32)
    h_pad[slots] = head_embeds[order]
    t_pad[slots] = tail_embeds[order]
    head_pad[slots] = head_list[order]

    h_pad = h_pad.reshape(NCORES, EPC, D)
    t_pad = t_pad.reshape(NCORES, EPC, D)
    head_pad = head_pad.reshape(NCORES, EPC)

    in_maps = [
        {"h": h_pad[i], "t": t_pad[i], "head": head_pad[i]}
        for i in range(NCORES)
    ]

    if "nc" not in _CACHE:
        _CACHE["nc"] = build_graph(P, K, C, D)
    nc = _CACHE["nc"]

    res = run_bass_kernel_spmd(nc, in_maps, core_ids=list(range(NCORES)))
    global LAST_EXEC_NS
    LAST_EXEC_NS = getattr(res, "exec_time_ns", None)
    g_flat = np.concatenate([res.results[i]["g"].reshape(-1) for i in range(NCORES)])

    g_values = np.empty(E, dtype=np.float32)
    g_values[order] = g_flat[slots]
    g_indices = np.stack([head_list, tail_list], axis=0)
    return g_indices, g_values


# revision 12
# speedup vs baseline: 1.3402x; 1.3402x over previous
"""Trainium2 8-core kernel for nn_AdaptiveMask (segment_reduce).

reference semantics:
    h = l2norm(head_embeds); t = l2norm(tail_embeds)
    edge_alpha = (sum(h*t, -1) + 1) * 0.5                       # [E]
    row_sum    = segment_sum(edge_alpha, head_list, N)          # [N]
    d_inv      = where(row_sum != 0, 1/row_sum, 0)
    G_indices  = stack([head_list, tail_list])                  # [2, E] i32
    G_values   = d_inv[head_list] * edge_alpha                  # [E] f32

Strategy (sharding = sort-pack permutation, inverted on output):
  * Host sorts edges by head (stable), packs whole segments (runs of equal
    head) into 8*128 partition rows of fixed capacity K, padding row tails
    with phantom edges.  No segment crosses a row, so every core/partition
    reduces its segments independently -- no collective needed.
  * Device (per core, SPMD identical graph):
      - DMA-cast embeds f32->bf16, chunked [128, C*64]
      - DVE: m_ht = h*t (bf16 2x); ACT: m_hh = h^2, m_tt = t^2
      - fold-pairs (bf16 2x adds) + tensor_reduce -> per-edge dots [128, C]
      - alpha = 0.5 + 0.5 * dht * rsqrt(dhh*dtt)   (ACT Sqrt + DVE recip)
      - segment masks from not_equal(head[e], head[e-1])
      - segmented prefix-sum scan + reversed hold scan (tensor_tensor_scan)
        give V[e] = row_sum[head_e] per edge, entirely in-SBUF
      - G[e] = alpha[e] * reciprocal(V[e])
  * Host scatters device output back to original edge order.
"""

import sys

for _p in ("/opt/trn_rl_repo",):
    if _p not in sys.path:
        sys.path.insert(0, _p)

import numpy as np

# ---------------------------------------------------------------- constants
E = 3_200_000
D = 64
N_NODES = 100_000
NCORES = 8
P = 128                 # SBUF partitions
K = 3200                # edge slots per partition row  (40 chunks of 80)
C = 80                  # edges per partition per chunk
ROWS = NCORES * P       # 1024 packing rows
EPC = P * K             # padded edges per core


def build_graph(p=P, k=K, c=C, d=D, repeat=1, stage="full"):
    """Build the single-core Bass graph (SPMD: all cores run this).

    repeat>1 wraps the whole body in a For_i loop (for HW timing only).
    """
    import concourse.bacc as bacc
    import concourse.mybir as mybir
    import concourse.tile as tile
    from contextlib import nullcontext

    f32 = mybir.dt.float32
    bf16 = mybir.dt.bfloat16
    i32 = mybir.dt.int32
    Alu = mybir.AluOpType
    Act = mybir.ActivationFunctionType

    nc = bacc.Bacc(None, target_bir_lowering=False)

    h_in = nc.dram_tensor("h", [p * k, d], f32, kind="ExternalInput")
    t_in = nc.dram_tensor("t", [p * k, d], f32, kind="ExternalInput")
    head_in = nc.dram_tensor("head", [p * k], i32, kind="ExternalInput")
    g_out = nc.dram_tensor("g", [p * k], f32, kind="ExternalOutput")

    hv = h_in[:].rearrange("(p k) d -> p (k d)", p=p)      # [P, K*D]
    tv = t_in[:].rearrange("(p k) d -> p (k d)", p=p)
    headv = head_in[:].rearrange("(p k) -> p k", p=p)      # [P, K]
    gv = g_out[:].rearrange("(p k) -> p k", p=p)

    nchunks = k // c

    with tile.TileContext(nc) as tc:
        with (
            tc.tile_pool(name="persist", bufs=1) as pers,
            tc.tile_pool(name="io", bufs=2) as io,
            tc.tile_pool(name="mbuf", bufs=1) as mb,
            tc.tile_pool(name="mbuf2", bufs=2) as mb2,
            tc.tile_pool(name="small", bufs=2) as sm,
            tc.tile_pool(name="folds", bufs=1) as fp,
            tc.For_i(0, repeat, 1) if repeat > 1 else nullcontext(),
        ):
            alpha = pers.tile([p, k], f32)
            notm0 = pers.tile([p, k + 1], f32)   # notm0[e]=1 iff head[e]==head[e-1]
            ps = pers.tile([p, k], f32, tag="pg")
            vrev = pers.tile([p, k], f32)
            headt = pers.tile([p, k], i32, tag="hs")

            nc.sync.dma_start(out=headt[:], in_=headv)

            # segment-start masks: notm0[:,0]=0 (row start), sentinel notm0[:,K]=0
            nc.vector.tensor_tensor(
                out=notm0[:, 1:k], in0=headt[:, 1:k], in1=headt[:, 0 : k - 1],
                op=Alu.is_equal,
            )
            nc.vector.memset(notm0[:, 0:1], 0.0)
            nc.vector.memset(notm0[:, k : k + 1], 0.0)

            for ci in range(nchunks):
                sl = slice(ci * c * d, (ci + 1) * c * d)
                hb = io.tile([p, c * d], bf16, tag="hb")
                tb = io.tile([p, c * d], bf16, tag="tb")
                nc.gpsimd.dma_start(out=hb[:], in_=hv[:, sl])   # f32->bf16 cast
                nc.gpsimd.dma_start(out=tb[:], in_=tv[:, sl])

                if stage == "dma":
                    continue
                mht = mb.tile([p, c * d], bf16, tag="mht")
                mhh = mb2.tile([p, c * d], bf16, tag="mhh")
                mtt = mb2.tile([p, c * d], bf16, tag="mtt")
                nc.scalar.activation(mhh[:], hb[:], Act.Square)
                nc.scalar.activation(mtt[:], tb[:], Act.Square)
                nc.vector.tensor_mul(mht[:], hb[:], tb[:])
                if stage == "mul":
                    continue

                f2a = sm.tile([p, 3, c, d // 4], bf16, tag="f2a")
                for si, m in enumerate((mhh, mtt, mht)):
                    m3 = m[:].rearrange("p (c d) -> p c d", d=d)
                    f1 = fp.tile([p, c, d // 2], bf16, tag="f1")
                    nc.vector.tensor_add(
                        f1[:], m3[:, :, 0 : d // 2], m3[:, :, d // 2 : d]
                    )
                    nc.vector.tensor_add(
                        f2a[:, si], f1[:, :, 0 : d // 4], f1[:, :, d // 4 : d // 2]
                    )
                dots = sm.tile([p, 3, c], f32, tag="dots")
                nc.vector.tensor_reduce(
                    dots[:], f2a[:], axis=mybir.AxisListType.X, op=Alu.add
                )
                q = sm.tile([p, c], f32, tag="q")
                nc.vector.tensor_mul(q[:], dots[:, 0], dots[:, 1])
                s = sm.tile([p, c], f32, tag="s")
                nc.scalar.activation(s[:], q[:], Act.Sqrt)
                r = sm.tile([p, c], f32, tag="r")
                nc.vector.reciprocal(r[:], s[:])
                tmp = sm.tile([p, c], f32, tag="tmp")
                nc.vector.tensor_mul(tmp[:], dots[:, 2], r[:])
                nc.vector.tensor_scalar(
                    out=alpha[:, ci * c : (ci + 1) * c], in0=tmp[:],
                    scalar1=0.5, scalar2=0.5, op0=Alu.mult, op1=Alu.add,
                )

            if stage in ("dma", "mul"):
                nc.vector.memset(alpha[:], 0.5)
            # ps[e] = notm0[e]*ps[e-1] + alpha[e]  (segmented prefix sum)
            nc.vector.tensor_tensor_scan(
                out=ps[:], data0=notm0[:, 0:k], data1=alpha[:],
                initial=0.0, op0=Alu.mult, op1=Alu.add,
            )
            # stamp[e] = ps[e] * m1[e],  m1[e] = 1 - notm0[e+1]  (segment ends)
            stamp = pers.tile([p, k], f32, tag="hs")
            nc.vector.tensor_mul(stamp[:], ps[:], notm0[:, 1 : k + 1])
            nc.vector.tensor_sub(stamp[:], ps[:], stamp[:])
            # reversed hold scan: V[e] = segment total broadcast over segment
            nc.vector.tensor_tensor_scan(
                out=vrev[:], data0=notm0[:, k:0:-1], data1=stamp[:, ::-1],
                initial=0.0, op0=Alu.mult, op1=Alu.add,
            )
            nc.vector.reciprocal(vrev[:], vrev[:])
            gtile = pers.tile([p, k], f32, tag="pg")
            nc.vector.tensor_mul(gtile[:], alpha[:], vrev[:, ::-1])
            nc.sync.dma_start(out=gv, in_=gtile[:])

    nc.finalize()
    return nc


# ---------------------------------------------------------------- host side

def _pack(head_list, k):
    """Sort edges by head; pack whole segments into ROWS rows of capacity k.

    Returns (order, slots) where order[i] = original edge id of i-th sorted
    edge and slots[i] = its slot in the padded [ROWS*k] device layout.
    Returns None on packing overflow (caller retries with larger k).
    """
    order = np.argsort(head_list, kind="stable")
    hs = head_list[order]
    # segment boundaries
    change = np.flatnonzero(hs[1:] != hs[:-1]) + 1
    seg_starts = np.concatenate([[0], change])
    seg_sizes = np.diff(np.concatenate([seg_starts, [len(hs)]]))
    cum = np.cumsum(seg_sizes)
    nseg = len(seg_sizes)
    if seg_sizes.max() > k:
        return None

    counts = np.zeros(ROWS, dtype=np.int64)
    base = 0
    j0 = 0
    for rr in range(ROWS):
        j = int(np.searchsorted(cum, base + k, side="right"))
        if j > j0:
            counts[rr] = cum[j - 1] - base
            base = cum[j - 1]
            j0 = j
        if j0 >= nseg:
            break
    if j0 < nseg:
        return None

    off = np.concatenate([[0], np.cumsum(counts)[:-1]])
    row_of = np.repeat(np.arange(ROWS), counts)
    pos_in_row = np.arange(len(hs)) - np.repeat(off, counts)
    slots = row_of * k + pos_in_row
    return order, slots


_CACHE = {}


def kernel(head_embeds, tail_embeds, head_list, tail_list):
    from concourse.bass_utils import run_bass_kernel_spmd

    head_embeds = np.asarray(head_embeds, dtype=np.float32)
    tail_embeds = np.asarray(tail_embeds, dtype=np.float32)
    head_list = np.asarray(head_list, dtype=np.int32)
    tail_list = np.asarray(tail_list, dtype=np.int32)

    k = K
    packed = _pack(head_list, k)
    while packed is None:        # denser data than expected: retry roomier
        k += 10 * C
        assert k <= 16 * K, "packing failed"
        packed = _pack(head_list, k)
    order, slots = packed

    epc = P * k
    h_pad = np.ones((ROWS * k, D), dtype=np.float32)
    t_pad = np.ones((ROWS * k, D), dtype=np.float32)
    head_pad = np.full(ROWS * k, N_NODES, dtype=np.int32)
    h_pad[slots] = head_embeds[order]
    t_pad[slots] = tail_embeds[order]
    head_pad[slots] = head_list[order]

    h_pad = h_pad.reshape(NCORES, epc, D)
    t_pad = t_pad.reshape(NCORES, epc, D)
    head_pad = head_pad.reshape(NCORES, epc)

    in_maps = [
        {"h": h_pad[i], "t": t_pad[i], "head": head_pad[i]}
        for i in range(NCORES)
    ]

    if k not in _CACHE:
        _CACHE[k] = build_graph(P, k, C, D)
    nc = _CACHE[k]

    res = run_bass_kernel_spmd(nc, in_maps, core_ids=list(range(NCORES)))
    global LAST_EXEC_NS
    LAST_EXEC_NS = getattr(res, "exec_time_ns", None)
    g_flat = np.concatenate([res.results[i]["g"].reshape(-1) for i in range(NCORES)])

    g_values = np.empty(E, dtype=np.float32)
    g_values[order] = g_flat[slots]
    g_indices = np.stack([head_list, tail_list], axis=0)
    return g_indices, g_values


# revision 16
# speedup vs baseline: 1.7932x; 1.3380x over previous
"""Trainium2 8-core kernel for nn_AdaptiveMask (segment_reduce).

reference semantics:
    h = l2norm(head_embeds); t = l2norm(tail_embeds)
    edge_alpha = (sum(h*t, -1) + 1) * 0.5                       # [E]
    row_sum    = segment_sum(edge_alpha, head_list, N)          # [N]
    d_inv      = where(row_sum != 0, 1/row_sum, 0)
    G_indices  = stack([head_list, tail_list])                  # [2, E] i32
    G_values   = d_inv[head_list] * edge_alpha                  # [E] f32

Strategy (sharding = sort-pack permutation, inverted on output):
  * Host sorts edges by head (stable), packs whole segments (runs of equal
    head) into 8*128 partition rows of fixed capacity K, padding row tails
    with phantom edges.  No segment crosses a row, so every core/partition
    reduces its segments independently -- no collective needed.
  * Device (per core, SPMD identical graph):
      - DMA-cast embeds f32->bf16, chunked [128, C*64]
      - DVE: m_ht = h*t (bf16 2x); ACT: m_hh = h^2, m_tt = t^2
      - fold-pairs (bf16 2x adds) + tensor_reduce -> per-edge dots [128, C]
      - alpha = 0.5 + 0.5 * dht * rsqrt(dhh*dtt)   (ACT Sqrt + DVE recip)
      - segment masks from not_equal(head[e], head[e-1])
      - segmented prefix-sum scan + reversed hold scan (tensor_tensor_scan)
        give V[e] = row_sum[head_e] per edge, entirely in-SBUF
      - G[e] = alpha[e] * reciprocal(V[e])
  * Host scatters device output back to original edge order.
"""

import sys

for _p in ("/opt/trn_rl_repo",):
    if _p not in sys.path:
        sys.path.insert(0, _p)

import numpy as np

# ---------------------------------------------------------------- constants
E = 3_200_000
D = 64
N_NODES = 100_000
NCORES = 8
P = 128                 # SBUF partitions
K = 3200                # edge slots per partition row  (40 chunks of 80)
C = 80                  # edges per partition per chunk
ROWS = NCORES * P       # 1024 packing rows
EPC = P * K             # padded edges per core


def build_graph(p=P, k=K, c=C, d=D, repeat=1, stage="full"):
    """Build the single-core Bass graph (SPMD: all cores run this).

    repeat>1 wraps the whole body in a For_i loop (for HW timing only).
    """
    import concourse.bacc as bacc
    import concourse.mybir as mybir
    import concourse.tile as tile
    from contextlib import nullcontext

    f32 = mybir.dt.float32
    bf16 = mybir.dt.bfloat16
    i32 = mybir.dt.int32
    Alu = mybir.AluOpType
    Act = mybir.ActivationFunctionType

    nc = bacc.Bacc(None, target_bir_lowering=False)

    h_in = nc.dram_tensor("h", [p * k, d], f32, kind="ExternalInput")
    t_in = nc.dram_tensor("t", [p * k, d], f32, kind="ExternalInput")
    head_in = nc.dram_tensor("head", [p * k], i32, kind="ExternalInput")
    g_out = nc.dram_tensor("g", [p * k], f32, kind="ExternalOutput")

    hv = h_in[:].rearrange("(p k) d -> p (k d)", p=p)      # [P, K*D]
    tv = t_in[:].rearrange("(p k) d -> p (k d)", p=p)
    headv = head_in[:].rearrange("(p k) -> p k", p=p)      # [P, K]
    gv = g_out[:].rearrange("(p k) -> p k", p=p)

    nchunks = k // c

    with tile.TileContext(nc) as tc:
        with (
            tc.tile_pool(name="persist", bufs=1) as pers,
            tc.tile_pool(name="io", bufs=2) as io,
            tc.tile_pool(name="mbuf", bufs=1) as mb,
            tc.tile_pool(name="mbuf2", bufs=2) as mb2,
            tc.tile_pool(name="small", bufs=2) as sm,
            tc.tile_pool(name="folds", bufs=1) as fp,
            tc.For_i(0, repeat, 1) if repeat > 1 else nullcontext(),
        ):
            alpha = pers.tile([p, k], f32)
            notm0 = pers.tile([p, k + 1], f32)   # notm0[e]=1 iff head[e]==head[e-1]
            ps = pers.tile([p, k], f32, tag="pg")
            vrev = pers.tile([p, k], f32)
            headt = pers.tile([p, k], i32, tag="hs")

            nc.sync.dma_start(out=headt[:], in_=headv)

            # segment-start masks: notm0[:,0]=0 (row start), sentinel notm0[:,K]=0
            nc.vector.tensor_tensor(
                out=notm0[:, 1:k], in0=headt[:, 1:k], in1=headt[:, 0 : k - 1],
                op=Alu.is_equal,
            )
            nc.vector.memset(notm0[:, 0:1], 0.0)
            nc.vector.memset(notm0[:, k : k + 1], 0.0)

            for ci in range(nchunks):
                sl = slice(ci * c * d, (ci + 1) * c * d)
                hb = io.tile([p, c * d], bf16, tag="hb")
                tb = io.tile([p, c * d], bf16, tag="tb")
                nc.gpsimd.dma_start(out=hb[:], in_=hv[:, sl])   # f32->bf16 cast
                nc.gpsimd.dma_start(out=tb[:], in_=tv[:, sl])

                if stage == "dma":
                    continue
                mht = mb.tile([p, c * d], bf16, tag="mht")
                mhh = mb2.tile([p, c * d], bf16, tag="mhh")
                mtt = mb2.tile([p, c * d], bf16, tag="mtt")
                nc.scalar.activation(mhh[:], hb[:], Act.Square)
                nc.scalar.activation(mtt[:], tb[:], Act.Square)
                nc.vector.tensor_mul(mht[:], hb[:], tb[:])
                if stage == "mul":
                    continue

                f2a = sm.tile([p, 3, c, d // 4], bf16, tag="f2a")
                for si, m in enumerate((mhh, mtt, mht)):
                    m3 = m[:].rearrange("p (c d) -> p c d", d=d)
                    f1 = fp.tile([p, c, d // 2], bf16, tag="f1")
                    nc.vector.tensor_add(
                        f1[:], m3[:, :, 0 : d // 2], m3[:, :, d // 2 : d]
                    )
                    nc.vector.tensor_add(
                        f2a[:, si], f1[:, :, 0 : d // 4], f1[:, :, d // 4 : d // 2]
                    )
                dots = sm.tile([p, 3, c], f32, tag="dots")
                nc.vector.tensor_reduce(
                    dots[:], f2a[:], axis=mybir.AxisListType.X, op=Alu.add
                )
                q = sm.tile([p, c], f32, tag="q")
                nc.vector.tensor_mul(q[:], dots[:, 0], dots[:, 1])
                s = sm.tile([p, c], f32, tag="s")
                nc.scalar.activation(s[:], q[:], Act.Sqrt)
                r = sm.tile([p, c], f32, tag="r")
                nc.vector.reciprocal(r[:], s[:])
                tmp = sm.tile([p, c], f32, tag="tmp")
                nc.vector.tensor_mul(tmp[:], dots[:, 2], r[:])
                nc.vector.tensor_scalar(
                    out=alpha[:, ci * c : (ci + 1) * c], in0=tmp[:],
                    scalar1=0.5, scalar2=0.5, op0=Alu.mult, op1=Alu.add,
                )

            if stage in ("dma", "mul"):
                nc.vector.memset(alpha[:], 0.5)
            # ps[e] = notm0[e]*ps[e-1] + alpha[e]  (segmented prefix sum)
            nc.vector.tensor_tensor_scan(
                out=ps[:], data0=notm0[:, 0:k], data1=alpha[:],
                initial=0.0, op0=Alu.mult, op1=Alu.add,
            )
            # stamp[e] = ps[e] * m1[e],  m1[e] = 1 - notm0[e+1]  (segment ends)
            stamp = pers.tile([p, k], f32, tag="hs")
            nc.vector.tensor_mul(stamp[:], ps[:], notm0[:, 1 : k + 1])
            nc.vector.tensor_sub(stamp[:], ps[:], stamp[:])
            # reversed hold scan: V[e] = segment total broadcast over segment
            nc.vector.tensor_tensor_scan(
                out=vrev[:], data0=notm0[:, k:0:-1], data1=stamp[:, ::-1],
                initial=0.0, op0=Alu.mult, op1=Alu.add,
            )
            nc.vector.reciprocal(vrev[:], vrev[:])
            gtile = pers.tile([p, k], f32, tag="pg")
            nc.vector.tensor_mul(gtile[:], alpha[:], vrev[:, ::-1])
            nc.sync.dma_start(out=gv, in_=gtile[:])

    nc.finalize()
    return nc


# ---------------------------------------------------------------- host side

def _pack(head_list, k):
    """Sort edges by head; pack whole segments into ROWS rows of capacity k.

    Returns (order, slots) where order[i] = original edge id of i-th sorted
    edge and slots[i] = its slot in the padded [ROWS*k] device layout.
    Returns None on packing overflow (caller retries with larger k).
    """
    order = np.argsort(head_list, kind="stable")
    hs = head_list[order]
    # segment boundaries
    change = np.flatnonzero(hs[1:] != hs[:-1]) + 1
    seg_starts = np.concatenate([[0], change])
    seg_sizes = np.diff(np.concatenate([seg_starts, [len(hs)]]))
    cum = np.cumsum(seg_sizes)
    nseg = len(seg_sizes)
    if seg_sizes.max() > k:
        return None

    counts = np.zeros(ROWS, dtype=np.int64)
    base = 0
    j0 = 0
    for rr in range(ROWS):
        j = int(np.searchsorted(cum, base + k, side="right"))
        if j > j0:
            counts[rr] = cum[j - 1] - base
            base = cum[j - 1]
            j0 = j
        if j0 >= nseg:
            break
    if j0 < nseg:
        return None

    off = np.concatenate([[0], np.cumsum(counts)[:-1]])
    row_of = np.repeat(np.arange(ROWS), counts)
    pos_in_row = np.arange(len(hs)) - np.repeat(off, counts)
    slots = row_of * k + pos_in_row
    return order, slots


_CACHE = {}


def kernel(head_embeds, tail_embeds, head_list, tail_list):
    from concourse.bass_utils import run_bass_kernel_spmd

    head_embeds = np.asarray(head_embeds, dtype=np.float32)
    tail_embeds = np.asarray(tail_embeds, dtype=np.float32)
    head_list = np.asarray(head_list, dtype=np.int32)
    tail_list = np.asarray(tail_list, dtype=np.int32)

    k = K
    packed = _pack(head_list, k)
    while packed is None:        # denser data than expected: retry roomier
        k += 10 * C
        assert k <= 16 * K, "packing failed"
        packed = _pack(head_list, k)
    order, slots = packed

    epc = P * k
    h_pad = np.ones((ROWS * k, D), dtype=np.float32)
    t_pad = np.ones((ROWS * k, D), dtype=np.float32)
    head_pad = np.full(ROWS * k, N_NODES, dtype=np.int32)
    h_pad[slots] = head_embeds[order]
    t_pad[slots] = tail_embeds[order]
    head_pad[slots] = head_list[order]

    h_pad = h_pad.reshape(NCORES, epc, D)
    t_pad = t_pad.reshape(NCORES, epc, D)
    head_pad = head_pad.reshape(NCORES, epc)

    in_maps = [
        {"h": h_pad[i], "t": t_pad[i], "head": head_pad[i]}
        for i in range(NCORES)
    ]

    if k not in _CACHE:
        _CACHE[k] = build_graph(P, k, C, D)
    nc = _CACHE[k]

    res = run_bass_kernel_spmd(nc, in_maps, core_ids=list(range(NCORES)))
    global LAST_EXEC_NS
    LAST_EXEC_NS = getattr(res, "exec_time_ns", None)
    g_flat = np.concatenate([res.results[i]["g"].reshape(-1) for i in range(NCORES)])

    g_values = np.empty(E, dtype=np.float32)
    g_values[order] = g_flat[slots]
    g_indices = np.stack([head_list, tail_list], axis=0)
    return g_indices, g_values
